# revision 1
# baseline (speedup 1.0000x reference)
"""Trainium2 Bass kernel for nn_DynamicConv2d: per-sample dynamic conv.

  feat = x.mean(H,W); h1 = relu(feat@w1+b1); wgen = (h1@w2+b2) -> per-sample
  [COUT, CIN, 3, 3] conv weights; out[s] = conv2d(x[s], wgen[s], pad=1).

Sharding: batch B=32 across 8 cores (4 samples/core), MLP params replicated.

Per-core pipeline:
  - x arrives host-width-padded [4, 64, 128, 130] (zero side cols); loaded as
    two sample-pair images xp [(sp,ci)=128 partitions, 128, 130] fp32
  - feat: chunked DVE free-dim reduces overlapped with the x DMA
  - h1T = matmul(lhsT=w1/(H*W), rhs=feat4) -> Relu+b1 on ScalarE -> bf16
  - wgen: 72 chunk matmuls (bf16 in, fp32 psum; stationary h1T), two
    column-group tile_position packs per PSUM fill, rhs AP ordered
    [co16, ci_low32] so a DVE StreamTranspose lands ci on partitions;
    strided cross-quadrant copies assemble block-diagonal conv weights
    WT2 [(sp,ci), (sp,co), o] (off-diagonal zeros); + b2 via gathered tile
  - conv: per offset o one [K=128, M=128, N<=512] fp32r matmul per pair-tile
    (block-diag stationary covers both samples), 9 offsets accumulate in one
    PSUM bank; ScalarE drains; DMA out.
"""

import sys

for _p in ("/opt/trn_rl_repo",):
    if _p not in sys.path:
        sys.path.insert(0, _p)

from contextlib import ExitStack

import numpy as np

import concourse.bass as bass
import concourse.tile as tile
from concourse import bacc, mybir
from concourse.bass_utils import run_bass_kernel_spmd

F32 = mybir.dt.float32
F32R = mybir.dt.float32r
BF16 = mybir.dt.bfloat16

B, CIN, COUT, K, H, W = 32, 64, 64, 3, 128, 128
NCORES = 8
BSH = B // NCORES          # 4 samples per core
NPAIR = BSH // 2           # 2 sample-pairs per core
HID = 128                  # MLP hidden
JTOT = COUT * CIN * K * K  # 36864
NOFF = K * K               # 9
HW = H * W
WP = W + 2                 # width-padded image


def build_kernel_body(nc, tc, ctx, aps):
    x_ap = aps["x"]      # [BSH, CIN, H, WP]  (host width-padded)
    w1_ap = aps["w1"]    # [CIN, HID]
    b1_ap = aps["b1"]    # [HID, 1]
    w2_ap = aps["w2"]    # [HID, JTOT]
    b2_ap = aps["b2"]    # [JTOT]
    out_ap = aps["out"]  # [BSH, COUT, H, W]

    const = ctx.enter_context(tc.tile_pool(name="const", bufs=1))
    xpool = ctx.enter_context(tc.tile_pool(name="xpool", bufs=2))
    w2pool = ctx.enter_context(tc.tile_pool(name="w2pool", bufs=1))
    tpool = ctx.enter_context(tc.tile_pool(name="tpool", bufs=1))
    wtpool = ctx.enter_context(tc.tile_pool(name="wtpool", bufs=2))
    fpool = ctx.enter_context(tc.tile_pool(name="fpool", bufs=4))
    outp = ctx.enter_context(tc.tile_pool(name="outp", bufs=4))
    mlp_ps = ctx.enter_context(tc.tile_pool(name="mlp_ps", bufs=1, space="PSUM"))
    wg_ps = ctx.enter_context(tc.tile_pool(name="wg_ps", bufs=3, space="PSUM"))
    cv_ps = ctx.enter_context(tc.tile_pool(name="cv_ps", bufs=4, space="PSUM"))

    # ---- constants ----
    w1_sb = const.tile([CIN, HID], F32)
    nc.sync.dma_start(out=w1_sb, in_=w1_ap)
    w1s = const.tile([CIN, HID], F32)
    nc.scalar.mul(out=w1s, in_=w1_sb, mul=1.0 / HW)
    b1_sb = const.tile([HID, 1], F32)
    nc.sync.dma_start(out=b1_sb, in_=b1_ap)

    # b2 arrives host-prepped in block-diagonal conv layout
    # [(sp,ci), (sp,co), o] -- one clean contiguous DMA.
    b2T2 = const.tile([2 * CIN, 2 * COUT, NOFF], F32)
    nc.sync.dma_start(out=b2T2, in_=b2_ap)

    # ---- x loads + feat partial reduces (pair 0 first, then w2, then pair 1
    # -- HWDGE FIFO order makes conv-pair0's inputs land first) ----
    NXC = 4  # sub-DMAs per pair
    rows_per = H // NXC
    x2 = x_ap.rearrange("s c h w -> (s c) h w")
    xp_tiles = [None] * NPAIR
    fsum4 = const.tile([2 * CIN, BSH], F32)

    def load_pair(p):
        xp = xpool.tile([2 * CIN, H, WP], BF16, tag="xp", name=f"xp{p}")
        xp_tiles[p] = xp
        for c in range(NXC):
            r0 = c * rows_per
            nc.sync.dma_start(
                out=xp[:, r0 : r0 + rows_per, :],
                in_=x2[2 * p * CIN : (2 * p + 2) * CIN, r0 : r0 + rows_per, :],
            )

    def feat_pair(p):
        # per-chunk channel sums on ScalarE (accum_out); keeps DVE free for
        # the wgen StreamTranspose on the critical path
        xp = xp_tiles[p]
        fpart = fpool.tile([2 * CIN, NXC], F32, tag="fpart", name=f"fpart{p}")
        for c in range(NXC):
            r0 = c * rows_per
            if c % 2 == 0:
                ascr = fpool.tile(
                    [2 * CIN, rows_per * W], BF16, tag="ascr", name=f"ascr{p}_{c}"
                )
                nc.scalar.activation(
                    out=ascr,
                    in_=xp[:, r0 : r0 + rows_per, 1 : W + 1],
                    func=mybir.ActivationFunctionType.Copy,
                    accum_out=fpart[:, c : c + 1],
                )
            else:
                nc.vector.tensor_reduce(
                    out=fpart[:, c : c + 1],
                    in_=xp[:, r0 : r0 + rows_per, 1 : W + 1],
                    axis=mybir.AxisListType.XY,
                    op=mybir.AluOpType.add,
                )
        nc.vector.tensor_reduce(
            out=fsum4[:, 2 * p : 2 * p + 1],
            in_=fpart,
            axis=mybir.AxisListType.X,
            op=mybir.AluOpType.add,
        )

    NCB = 4
    CO_SL = COUT // NCB  # 16 co per slice
    SL = CO_SL * CIN * NOFF  # 9216
    COH = CO_SL // 2  # 8: co-half within a slice
    NW = COH * 32  # 256: chunk width (co-half x ci_low32)

    load_pair(0)
    load_pair(1)
    feat_pair(0)
    feat_pair(1)
    w2sl_tiles = []
    for cb in range(NCB):
        w2sl = w2pool.tile([HID, SL], BF16, tag=f"w2sl{cb}", name=f"w2sl{cb}")
        nc.sync.dma_start(out=w2sl, in_=w2_ap[:, cb * SL : (cb + 1) * SL])
        w2sl_tiles.append(w2sl)

    # ---- MLP (all 4 samples): h1T = relu(w1s.T @ feat4 + b1) -> bf16 ----
    feat4 = const.tile([CIN, BSH], F32)
    for p in range(NPAIR):
        nc.vector.tensor_copy(
            out=feat4[:, 2 * p : 2 * p + 1], in_=fsum4[0:CIN, 2 * p : 2 * p + 1]
        )
        nc.vector.tensor_copy(
            out=feat4[:, 2 * p + 1 : 2 * p + 2],
            in_=fsum4[CIN : 2 * CIN, 2 * p : 2 * p + 1],
        )
    h1_ps = mlp_ps.tile([HID, BSH], F32)
    nc.tensor.matmul(out=h1_ps, lhsT=w1s, rhs=feat4, start=True, stop=True)
    h1T32 = const.tile([HID, 32], BF16)
    nc.vector.memset(h1T32, 0.0)
    nc.scalar.activation(
        out=h1T32[:, 0:BSH],
        in_=h1_ps,
        func=mybir.ActivationFunctionType.Relu,
        bias=b1_sb,
        scale=1.0,
    )

    # ---- wgen: 4 column-groups per fill (2 co-blocks x 2 ci-halves) so the
    # StreamTranspose runs dense 128-partition ops ----
    wt_tiles = []
    for p in range(NPAIR):
        wt = wtpool.tile([2 * CIN, 2 * COUT, NOFF], BF16, tag="wt", name=f"wt{p}")
        wt_tiles.append(wt)
        nc.vector.memset(wt, 0.0)

    for cb in range(NCB):
        w2r = w2sl_tiles[cb].rearrange(
            "h (co ci o) -> h co ci o", co=CO_SL, ci=CIN, o=NOFF
        )
        tmid = tpool.tile(
            [2 * CIN, NOFF * NW], F32, tag="tmid", name=f"tmid{cb}"
        )
        for o in range(NOFF):
            wps = wg_ps.tile([2 * CIN, NW], F32, tag="wps", name=f"wps{cb}_{o}")
            for g in range(4):  # (co-half, ci-half)
                h, cih = g // 2, g % 2
                nc.tensor.matmul(
                    out=wps[32 * g : 32 * (g + 1), :],
                    lhsT=h1T32,
                    rhs=w2r[
                        :, COH * h : COH * (h + 1), 32 * cih : 32 * (cih + 1), o
                    ],
                    start=True,
                    stop=True,
                    tile_position=(0, 32 * g),
                )
            # T[32g + cil, 32co + s] = wps[32g + s, 32co + cil]
            nc.vector.transpose(out=tmid[:, o * NW : (o + 1) * NW], in_=wps)
        tr = tmid.rearrange("p (o co s) -> p o co s", o=NOFF, co=COH, s=32)
        for s in range(BSH):
            pr, sp = s // 2, s % 2
            for h in range(2):
                dst = wt_tiles[pr][
                    sp * CIN : (sp + 1) * CIN,
                    sp * COUT + cb * CO_SL + h * COH :
                    sp * COUT + cb * CO_SL + (h + 1) * COH,
                    :,
                ].rearrange("p co o -> p o co")
                nc.vector.tensor_copy(
                    out=dst, in_=tr[64 * h : 64 * (h + 1), :, :, s]
                )

    for p in range(NPAIR):
        nc.vector.tensor_add(wt_tiles[p], wt_tiles[p], b2T2)

    # ---- conv ----
    TROWS = 4
    NT = H // TROWS
    out2 = out_ap.rearrange("s c h w -> (s c) (h w)")
    # center offset first so start=True covers every psum element
    off_order = [4, 0, 1, 2, 3, 5, 6, 7, 8]
    for p in range(NPAIR):
        wt = wt_tiles[p]
        xr = xp_tiles[p]  # [q, H, WP]
        TB = 2  # conv tiles batched per output DMA
        for tb in range(NT // TB):
            ost = outp.tile(
                [2 * CIN, TB * TROWS * W], F32, tag="ost", name=f"ost{p}_{tb}"
            )
            for tt in range(TB):
                t = tb * TB + tt
                h0 = t * TROWS
                cvp = cv_ps.tile(
                    [2 * CIN, TROWS * W], F32, tag="cvp", name=f"cvp{p}_{t}"
                )
                for i, o in enumerate(off_order):
                    dy, dx = o // 3, o % 3
                    h_lo = max(h0, 1 - dy)
                    h_hi = min(h0 + TROWS, H + 1 - dy)
                    nr = h_hi - h_lo
                    xr0 = h_lo + dy - 1
                    nc.tensor.matmul(
                        out=cvp[:, (h_lo - h0) * W : (h_hi - h0) * W],
                        lhsT=wt[:, :, o],
                        rhs=xr[:, xr0 : xr0 + nr, dx : dx + W],
                        start=(i == 0),
                        stop=(i == len(off_order) - 1),
                    )
                nc.scalar.copy(
                    out=ost[:, tt * TROWS * W : (tt + 1) * TROWS * W], in_=cvp
                )
            nc.sync.dma_start(
                out=out2[
                    2 * p * CIN : (2 * p + 2) * CIN,
                    tb * TB * TROWS * W : (tb + 1) * TB * TROWS * W,
                ],
                in_=ost,
            )


_CACHE = {}


def build_nc():
    if "nc" in _CACHE:
        return _CACHE["nc"], _CACHE["aps"]
    nc = bacc.Bacc("TRN2", debug=False, num_devices=NCORES)
    aps = {
        "x": nc.dram_tensor("x", [BSH, CIN, H, WP], BF16, kind="ExternalInput").ap(),
        "w1": nc.dram_tensor("w1", [CIN, HID], F32, kind="ExternalInput").ap(),
        "b1": nc.dram_tensor("b1", [HID, 1], F32, kind="ExternalInput").ap(),
        "w2": nc.dram_tensor("w2", [HID, JTOT], BF16, kind="ExternalInput").ap(),
        "b2": nc.dram_tensor(
            "b2", [2 * CIN, 2 * COUT, NOFF], F32, kind="ExternalInput"
        ).ap(),
        "out": nc.dram_tensor("out", [BSH, COUT, H, W], F32, kind="ExternalOutput").ap(),
    }
    with tile.TileContext(nc) as tc, ExitStack() as ctx:
        build_kernel_body(nc, tc, ctx, aps)
    nc.compile()
    _CACHE["nc"] = nc
    _CACHE["aps"] = aps
    return nc, aps


def make_in_maps(x, w1, b1, w2, b2):
    import ml_dtypes
    x = np.asarray(x, dtype=np.float32)
    xpad = np.zeros((B, CIN, H, WP), dtype=ml_dtypes.bfloat16)
    xpad[:, :, :, 1 : W + 1] = x.astype(ml_dtypes.bfloat16)
    w1 = np.ascontiguousarray(np.asarray(w1, dtype=np.float32))
    b1 = np.ascontiguousarray(np.asarray(b1, dtype=np.float32)).reshape(HID, 1)
    w2 = np.ascontiguousarray(
        np.asarray(w2, dtype=np.float32).astype(ml_dtypes.bfloat16)
    )
    b2v = np.asarray(b2, dtype=np.float32).reshape(COUT, CIN, NOFF)
    b2t = np.zeros((2 * CIN, 2 * COUT, NOFF), dtype=np.float32)
    for sp in range(2):
        b2t[sp * CIN : (sp + 1) * CIN, sp * COUT : (sp + 1) * COUT, :] = (
            b2v.transpose(1, 0, 2)
        )
    b2 = np.ascontiguousarray(b2t)
    in_maps = []
    for c in range(NCORES):
        in_maps.append(
            {
                "x": np.ascontiguousarray(xpad[c * BSH : (c + 1) * BSH]),
                "w1": w1,
                "b1": b1,
                "w2": w2,
                "b2": b2,
            }
        )
    return in_maps


def kernel(x, w1, b1, w2, b2, _trace=False, _results_out=None):
    nc, _ = build_nc()
    in_maps = make_in_maps(x, w1, b1, w2, b2)
    res = run_bass_kernel_spmd(
        nc, in_maps, core_ids=list(range(NCORES)), trace=_trace
    )
    if _results_out is not None:
        _results_out.append(res)
    out = np.concatenate([r["out"] for r in res.results], axis=0)
    return out


if __name__ == "__main__":
    rng = np.random.default_rng(0)
    ins = {
        "x": rng.standard_normal((B, CIN, H, W)).astype(np.float32),
        "w1": (rng.standard_normal((CIN, HID)) * 0.05).astype(np.float32),
        "b1": (rng.standard_normal((HID,)) * 0.05).astype(np.float32),
        "w2": (rng.standard_normal((HID, JTOT)) * 0.05).astype(np.float32),
        "b2": (rng.standard_normal((JTOT,)) * 0.05).astype(np.float32),
    }
    out = kernel(**ins)
    print("out", out.shape, out.dtype, np.abs(out).mean())



# revision 12
# speedup vs baseline: 1.4102x; 1.4102x over previous
"""Trainium2 Bass kernel for nn_DynamicConv2d: per-sample dynamic conv.

  feat = x.mean(H,W); h1 = relu(feat@w1+b1); wgen = (h1@w2+b2) -> per-sample
  [COUT, CIN, 3, 3] conv weights; out[s] = conv2d(x[s], wgen[s], pad=1).

Sharding: batch B=32 across 8 cores (4 samples/core), MLP params replicated.

Per-core pipeline (v2 -- x-stationary conv):
  - x arrives host zero-padded [4, 64, 130, 130] bf16; per sample an SBUF
    tile xd [128=(j,ci), 130, 130] holds j0 = padded image (DMA) and
    j1[r] = j0[r+1] (row-shifted dup, built by on-chip partition-shifted
    copies on ACT/Pool/DVE, overlapped with the x DMA)
  - feat: s0 via ACT copy+accum (the dup copy doubles as the reduction);
    s1-3 via DVE halving add-trees (level-1 bf16 at 4x, then fp32)
  - MLP: h1 = relu(w1.T/(HW*16) @ feat + b1/16) -> bf16 [128, 4]
  - wgen: w2 host-reordered/scaled(x16)/fp8 as stationary chunks
    [HID, 128]; rhs = h1 [HID, 4] -> psum lands directly in conv-weight
    layout [(dy,ci) | ci, (dx,co)]; DVE adds b2 -> bf16 wt tiles
  - conv: per (sample, row y): 6 matmuls into one psum [128pix, 64co]
    slice: 3 paired passes (K=128 contracts dy=0,1 x ci via the dup) +
    3 singles (dy=2, K=64 on the j1 half); 8 rows/psum bank; ACT drains
    bf16; DMA out in [s, yb, x, yr, co] layout, host restores NCHW fp32.
"""

import sys

for _p in ("/opt/trn_rl_repo",):
    if _p not in sys.path:
        sys.path.insert(0, _p)

from contextlib import ExitStack

import numpy as np

import concourse.bass as bass
import concourse.tile as tile
from concourse import bacc, mybir
from concourse.bass_utils import run_bass_kernel_spmd

F32 = mybir.dt.float32
BF16 = mybir.dt.bfloat16
F8 = mybir.dt.float8e4

B, CIN, COUT, K, H, W = 32, 64, 64, 3, 128, 128
NCORES = 8
BSH = B // NCORES          # 4 samples per core
HID = 128                  # MLP hidden
JTOT = COUT * CIN * K * K  # 36864
HW = H * W
RP, CP = H + 2, W + 2      # padded image dims
SCL = 16.0                 # fp8 w2 pre-scale (host mul, folded out via w1/b1)

NPAIRED = 2 * CIN * 3 * COUT   # paired-region w2 cols: (dx,co) x (j,ci)
NSING = CIN * 3 * COUT         # singles-region w2 cols: (dx,co) x ci
NT_P = NPAIRED // 128          # 192 paired chunks
NT_S = NSING // 64             # 192 single chunks
W2CH_P = 4096                  # paired DMA chunk cols (32 mm-chunks)
W2CH_S = 4096                  # singles DMA chunk cols (64 mm-chunks)

XROWS = [(0, 40), (40, 80), (80, 120), (120, 130)]  # x DMA row chunks


def build_kernel_body(nc, tc, ctx, aps):
    x_ap = aps["x"]        # [BSH, CIN, RP, CP] bf16 (host zero-padded)
    w1_ap = aps["w1"]      # [CIN, HID] f32
    b1_ap = aps["b1"]      # [HID, 1] f32 (host /SCL)
    w2_ap = aps["w2"]      # [HID, JTOT] fp8 (host reorder + *SCL)
    b2a_ap = aps["b2a"]    # [128, 192] f32  (j*64+ci, dx*64+co)
    b2b_ap = aps["b2b"]    # [64, 192] f32   (ci, dx*64+co)
    out_ap = aps["out"]    # [BSH, 16, 128, 8, 64] bf16

    const = ctx.enter_context(tc.tile_pool(name="const", bufs=1))
    xpool = ctx.enter_context(tc.tile_pool(name="xpool", bufs=1))
    w2pool = ctx.enter_context(tc.tile_pool(name="w2pool", bufs=9))
    tpool = ctx.enter_context(tc.tile_pool(name="tpool", bufs=2))
    outp = ctx.enter_context(tc.tile_pool(name="outp", bufs=4))
    wg_ps = ctx.enter_context(tc.tile_pool(name="wg_ps", bufs=4, space="PSUM"))
    cv_ps = ctx.enter_context(tc.tile_pool(name="cv_ps", bufs=4, space="PSUM"))

    # ---- constants ----
    w1_sb = const.tile([CIN, HID], F32)
    nc.sync.dma_start(out=w1_sb, in_=w1_ap)
    w1s = const.tile([CIN, HID], F32)
    nc.scalar.mul(out=w1s, in_=w1_sb, mul=1.0 / (HW * SCL))
    b1_sb = const.tile([HID, 1], F32)
    nc.sync.dma_start(out=b1_sb, in_=b1_ap)
    b2a = const.tile([128, 192], F32)
    nc.sync.dma_start(out=b2a, in_=b2a_ap)
    b2b = const.tile([CIN, 192], F32)
    nc.sync.dma_start(out=b2b, in_=b2b_ap)

    # ---- x DMA (round-robin over samples per row-chunk) ----
    xd = [xpool.tile([128, RP, CP], BF16, name=f"xd{s}") for s in range(BSH)]
    # tree samples (s2, s3) land first in each round so the DVE trees start
    # early; ACT-accum samples (s0, s1) have more slack
    for r0, r1 in XROWS:
        for s in (2, 0, 3, 1):
            nc.sync.dma_start(
                out=xd[s][0:CIN, r0:r1, :], in_=x_ap[s, :, r0:r1, :]
            )

    # ---- w2 DMA (paired region then singles) ----
    w2p_tiles = []
    for c in range(NPAIRED // W2CH_P):  # 6
        t = w2pool.tile([HID, W2CH_P], F8, tag="w2", name=f"w2p{c}")
        nc.sync.dma_start(
            out=t, in_=w2_ap[:, c * W2CH_P : (c + 1) * W2CH_P]
        )
        w2p_tiles.append(t)
    w2s_tiles = []
    for c in range(NSING // W2CH_S):  # 3
        t = w2pool.tile([HID, W2CH_S], F8, tag="w2", name=f"w2s{c}")
        nc.sync.dma_start(
            out=t,
            in_=w2_ap[:, NPAIRED + c * W2CH_S : NPAIRED + (c + 1) * W2CH_S],
        )
        w2s_tiles.append(t)

    # ---- feat + dup per sample ----
    feat4 = const.tile([CIN, BSH], F32)
    # s0: ACT copy+accum -- dup copy is also the feat reduction
    fp0 = const.tile([CIN, len(XROWS)], F32)
    for c, (r0, r1) in enumerate(XROWS):
        d0 = max(0, r0 - 1)
        d1 = r1 - 1
        nc.scalar.activation(
            out=xd[0][CIN : 2 * CIN, d0:d1, :],
            in_=xd[0][0:CIN, d0 + 1 : d1 + 1, :],
            func=mybir.ActivationFunctionType.Copy,
            accum_out=fp0[:, c : c + 1],
        )
    # s1: feat via DVE cascade below; dup via one DVE 4x copy after the MLP
    # s2+s3: Pool dup copies (feat via DVE trees below); s3's conv is last,
    # so its dup has until ~100us
    for sd in (2, 3):
        for c, (r0, r1) in enumerate(XROWS):
            d0 = max(0, r0 - 1)
            d1 = r1 - 1
            nc.gpsimd.tensor_copy(
                out=xd[sd][CIN : 2 * CIN, d0:d1, :],
                in_=xd[sd][0:CIN, d0 + 1 : d1 + 1, :],
            )
    # s1-3 feat: per-chunk bf16 halving cascade L1->L2->L3 on DVE (chases the
    # x DMA), L3 outputs concatenate per sample, one final reduce each
    def chunk_depth(n0):
        # halving levels until the width goes odd (keeps the cascade exact)
        d = 0
        while n0 % 2 == 0 and d < 3:
            n0 //= 2
            d += 1
        return d, n0

    cat_off = []
    off = 0
    for r0, r1 in XROWS:
        d, nf = chunk_depth((r1 - r0) * CP)
        cat_off.append(off)
        off += nf
    t3cat = {
        s: const.tile([CIN, off], BF16, name=f"t3cat{s}") for s in (1, 2, 3)
    }
    for c, (r0, r1) in enumerate(XROWS):
        n0 = (r1 - r0) * CP
        depth, _ = chunk_depth(n0)
        for s in (2, 3, 1):
            cur = xd[s][0:CIN, r0:r1, :].rearrange("p r c -> p (r c)")
            n = n0
            for lvl in range(depth):
                n //= 2
                if lvl == depth - 1:
                    dst = t3cat[s][:, cat_off[c] : cat_off[c] + n]
                else:
                    dst = tpool.tile(
                        [CIN, n], BF16, tag=f"tr{lvl}", name=f"t{lvl}_{s}_{c}"
                    )
                nc.vector.tensor_tensor(
                    out=dst, in0=cur[:, 0:n], in1=cur[:, n : 2 * n],
                    op=mybir.AluOpType.add,
                )
                cur = dst
    # finals: s1 on ACT (accum to scratch), s2/s3 on DVE, s0 from fp0
    s1scr = tpool.tile([CIN, off], BF16, tag="tr0", name="s1scr")
    nc.scalar.activation(
        out=s1scr, in_=t3cat[1], func=mybir.ActivationFunctionType.Copy,
        accum_out=feat4[:, 1:2],
    )
    for s in (2, 3):
        nc.vector.tensor_reduce(
            out=feat4[:, s : s + 1], in_=t3cat[s], axis=mybir.AxisListType.X,
            op=mybir.AluOpType.add,
        )
    nc.vector.tensor_reduce(
        out=feat4[:, 0:1], in_=fp0, axis=mybir.AxisListType.X,
        op=mybir.AluOpType.add,
    )

    # ---- MLP ----
    mlp_ps = wg_ps.tile([HID, BSH], F32, tag="wgps", name="mlp_ps")
    nc.tensor.matmul(out=mlp_ps, lhsT=w1s, rhs=feat4, start=True, stop=True)
    h1T = const.tile([HID, BSH], BF16)
    nc.scalar.activation(
        out=h1T, in_=mlp_ps, func=mybir.ActivationFunctionType.Relu,
        bias=b1_sb, scale=1.0,
    )

    # ---- wgen: w2 chunks stationary, h1 moving; psum lands in conv layout --
    psA = [
        wg_ps.tile([128, 512], F32, tag="wgps", name="psA0"),
        wg_ps.tile([128, 512], F32, tag="wgps", name="psA1"),
    ]
    psB = [
        wg_ps.tile([CIN, 512], F32, tag="wgps", name="psB0"),
        wg_ps.tile([CIN, 512], F32, tag="wgps", name="psB1"),
    ]
    for t in range(NT_P):  # paired: chunk t = dx*64+co, partitions (j,ci)
        buf = w2p_tiles[t // 32]
        off = (t % 32) * 128
        dst = psA[t // 128]
        j = (t % 128) * 4
        nc.tensor.matmul(
            out=dst[:, j : j + 4], lhsT=buf[:, off : off + 128], rhs=h1T,
            start=True, stop=True,
        )
    for u in range(NT_S):  # singles: chunk u = dx*64+co, partitions ci
        buf = w2s_tiles[u // 64]
        off = (u % 64) * 64
        dst = psB[u // 128]
        j = (u % 128) * 4
        nc.tensor.matmul(
            out=dst[:, j : j + 4], lhsT=buf[:, off : off + 64], rhs=h1T,
            start=True, stop=True,
        )

    # ---- wt assembly: bf16 wt = psum + b2 (DVE), conv-ready layout ----
    # wtab[s]: cols 0:192 = wtA [(j,ci), (dx,co)]; cols 192:384 rows 64:128
    #          = wtB [(ci)@base64, (dx,co)]
    wtab = [const.tile([128, 384], BF16, name=f"wtab{s}") for s in range(BSH)]
    pa0 = psA[0].rearrange("p (t f) -> p t f", f=4)
    pa1 = psA[1].rearrange("p (t f) -> p t f", f=4)
    pb0 = psB[0].rearrange("p (t f) -> p t f", f=4)
    pb1 = psB[1].rearrange("p (t f) -> p t f", f=4)

    def wt_adds(s):
        nc.vector.tensor_tensor(
            out=wtab[s][:, 0:128], in0=pa0[:, :, s], in1=b2a[:, 0:128],
            op=mybir.AluOpType.add,
        )
        nc.vector.tensor_tensor(
            out=wtab[s][:, 128:192], in0=pa1[:, 0:64, s], in1=b2a[:, 128:192],
            op=mybir.AluOpType.add,
        )
        nc.vector.tensor_tensor(
            out=wtab[s][CIN : 2 * CIN, 192:320], in0=pb0[:, :, s],
            in1=b2b[:, 0:128], op=mybir.AluOpType.add,
        )
        nc.vector.tensor_tensor(
            out=wtab[s][CIN : 2 * CIN, 320:384], in0=pb1[:, 0:64, s],
            in1=b2b[:, 128:192], op=mybir.AluOpType.add,
        )

    wt_adds(0)
    # s1 dup: one DVE 4x copy (deadline: s1's conv at ~60us)
    nc.vector.tensor_copy(
        out=xd[1][CIN : 2 * CIN, 0 : RP - 1, :],
        in_=xd[1][0:CIN, 1:RP, :],
    )
    for s in range(1, BSH):
        wt_adds(s)

    # ---- conv ----
    YR = 8  # rows per psum bank
    NYB = H // YR
    for s in range(BSH):
        for yb in range(NYB):
            cvp = cv_ps.tile([128, YR * COUT], F32, tag="cvp", name=f"cvp{s}_{yb}")
            for yr in range(YR):
                y = yb * YR + yr
                sl = cvp[:, yr * COUT : (yr + 1) * COUT]
                for i, dx in enumerate((0, 1, 2)):
                    nc.tensor.matmul(
                        out=sl,
                        lhsT=xd[s][:, y, dx : dx + 128],
                        rhs=wtab[s][:, 64 * dx : 64 * dx + 64],
                        start=(i == 0), stop=False,
                    )
                for i, dx in enumerate((0, 1, 2)):
                    nc.tensor.matmul(
                        out=sl,
                        lhsT=xd[s][CIN : 2 * CIN, y + 1, dx : dx + 128],
                        rhs=wtab[s][CIN : 2 * CIN, 192 + 64 * dx : 256 + 64 * dx],
                        start=False, stop=(i == 2),
                    )
            ost = outp.tile([128, YR * COUT], BF16, tag="ost", name=f"ost{s}_{yb}")
            nc.scalar.copy(out=ost, in_=cvp)
            nc.sync.dma_start(out=out_ap[s, yb], in_=ost)


_CACHE = {}


def build_nc():
    if "nc" in _CACHE:
        return _CACHE["nc"], _CACHE["aps"]
    nc = bacc.Bacc("TRN2", debug=False, num_devices=NCORES)
    aps = {
        "x": nc.dram_tensor("x", [BSH, CIN, RP, CP], BF16, kind="ExternalInput").ap(),
        "w1": nc.dram_tensor("w1", [CIN, HID], F32, kind="ExternalInput").ap(),
        "b1": nc.dram_tensor("b1", [HID, 1], F32, kind="ExternalInput").ap(),
        "w2": nc.dram_tensor("w2", [HID, JTOT], F8, kind="ExternalInput").ap(),
        "b2a": nc.dram_tensor("b2a", [128, 192], F32, kind="ExternalInput").ap(),
        "b2b": nc.dram_tensor("b2b", [CIN, 192], F32, kind="ExternalInput").ap(),
        "out": nc.dram_tensor(
            "out", [BSH, H // 8, 128, 8, COUT], BF16, kind="ExternalOutput"
        ).ap(),
    }
    with tile.TileContext(nc) as tc, ExitStack() as ctx:
        build_kernel_body(nc, tc, ctx, aps)
    nc.compile()
    _CACHE["nc"] = nc
    _CACHE["aps"] = aps
    return nc, aps


def make_in_maps(x, w1, b1, w2, b2):
    import ml_dtypes

    x = np.asarray(x, dtype=np.float32)
    xpad = np.zeros((B, CIN, RP, CP), dtype=ml_dtypes.bfloat16)
    xpad[:, :, 1 : H + 1, 1 : W + 1] = x.astype(ml_dtypes.bfloat16)
    w1 = np.ascontiguousarray(np.asarray(w1, dtype=np.float32))
    b1 = np.ascontiguousarray(
        np.asarray(b1, dtype=np.float32).reshape(HID, 1) / SCL
    )
    # w2 reorder: paired cols (dx,co)x(j,ci) from o=3j+dx; singles from o=6+dx
    w2v = (np.asarray(w2, dtype=np.float32) * SCL).reshape(HID, COUT, CIN, 9)
    w2p = w2v[:, :, :, [3 * j + dx for dx in range(3) for j in range(2)]]
    # -> [HID, co, ci, (dx,j)] want cols ((dx,co),(j,ci))
    w2p = w2p.reshape(HID, COUT, CIN, 3, 2).transpose(0, 3, 1, 4, 2)
    w2p = w2p.reshape(HID, NPAIRED)
    w2s = w2v[:, :, :, [6 + dx for dx in range(3)]].transpose(0, 3, 1, 2)
    w2s = w2s.reshape(HID, NSING)
    w2x = np.ascontiguousarray(
        np.concatenate([w2p, w2s], axis=1).astype(ml_dtypes.float8_e4m3fn)
    )
    b2v = np.asarray(b2, dtype=np.float32).reshape(COUT, CIN, 9)
    b2a = np.zeros((128, 192), dtype=np.float32)
    b2bt = np.zeros((CIN, 192), dtype=np.float32)
    for dx in range(3):
        for j in range(2):
            # b2a[j*64+ci, dx*64+co] = b2[co, ci, 3j+dx]
            b2a[j * CIN : (j + 1) * CIN, dx * COUT : (dx + 1) * COUT] = b2v[
                :, :, 3 * j + dx
            ].T
        b2bt[:, dx * COUT : (dx + 1) * COUT] = b2v[:, :, 6 + dx].T
    in_maps = []
    for c in range(NCORES):
        in_maps.append(
            {
                "x": np.ascontiguousarray(xpad[c * BSH : (c + 1) * BSH]),
                "w1": w1,
                "b1": b1,
                "w2": w2x,
                "b2a": np.ascontiguousarray(b2a),
                "b2b": np.ascontiguousarray(b2bt),
            }
        )
    return in_maps


def kernel(x, w1, b1, w2, b2, _trace=False, _results_out=None):
    nc, _ = build_nc()
    in_maps = make_in_maps(x, w1, b1, w2, b2)
    res = run_bass_kernel_spmd(
        nc, in_maps, core_ids=list(range(NCORES)), trace=_trace
    )
    if _results_out is not None:
        _results_out.append(res)
    # [BSH,16,128,8,64] bf16 per core -> [B, COUT, H, W] f32
    parts = []
    for r in res.results:
        o = np.asarray(r["out"], dtype=np.float32)  # [BSH,16,x128,yr8,co64]
        o = o.transpose(0, 4, 1, 3, 2).reshape(BSH, COUT, H, W)
        parts.append(o)
    return np.concatenate(parts, axis=0)


if __name__ == "__main__":
    rng = np.random.default_rng(0)
    ins = {
        "x": rng.standard_normal((B, CIN, H, W)).astype(np.float32),
        "w1": (rng.standard_normal((CIN, HID)) * 0.05).astype(np.float32),
        "b1": (rng.standard_normal((HID,)) * 0.05).astype(np.float32),
        "w2": (rng.standard_normal((HID, JTOT)) * 0.05).astype(np.float32),
        "b2": (rng.standard_normal((JTOT,)) * 0.05).astype(np.float32),
    }
    out = kernel(**ins)
    print("out", out.shape, out.dtype, np.abs(out).mean())


# revision 16
# speedup vs baseline: 1.4959x; 1.0608x over previous
"""Trainium2 Bass kernel for nn_DynamicConv2d: per-sample dynamic conv.

  feat = x.mean(H,W); h1 = relu(feat@w1+b1); wgen = (h1@w2+b2) -> per-sample
  [COUT, CIN, 3, 3] conv weights; out[s] = conv2d(x[s], wgen[s], pad=1).

Sharding: batch B=32 across 8 cores (4 samples/core), MLP params replicated.

Per-core pipeline (v2 -- x-stationary conv):
  - x arrives host zero-padded [4, 64, 130, 130] bf16; per sample an SBUF
    tile xd [128=(j,ci), 130, 130] holds j0 = padded image (DMA) and
    j1[r] = j0[r+1] (row-shifted dup, built by on-chip partition-shifted
    copies on ACT/Pool/DVE, overlapped with the x DMA)
  - feat: s0 via ACT copy+accum (the dup copy doubles as the reduction);
    s1-3 via DVE halving add-trees (level-1 bf16 at 4x, then fp32)
  - MLP: h1 = relu(w1.T/(HW*16) @ feat + b1/16) -> bf16 [128, 4]
  - wgen: w2 host-reordered/scaled(x16)/fp8 as stationary chunks
    [HID, 128]; rhs = h1 [HID, 4] -> psum lands directly in conv-weight
    layout [(dy,ci) | ci, (dx,co)]; DVE adds b2 -> bf16 wt tiles
  - conv: per (sample, row y): 6 matmuls into one psum [128pix, 64co]
    slice: 3 paired passes (K=128 contracts dy=0,1 x ci via the dup) +
    3 singles (dy=2, K=64 on the j1 half); 8 rows/psum bank; ACT drains
    bf16; DMA out in [s, yb, x, yr, co] layout, host restores NCHW fp32.
"""

import sys

for _p in ("/opt/trn_rl_repo",):
    if _p not in sys.path:
        sys.path.insert(0, _p)

from contextlib import ExitStack

import numpy as np

import concourse.bass as bass
import concourse.tile as tile
from concourse import bacc, mybir
from concourse.bass_utils import run_bass_kernel_spmd

F32 = mybir.dt.float32
BF16 = mybir.dt.bfloat16
F8 = mybir.dt.float8e4

B, CIN, COUT, K, H, W = 32, 64, 64, 3, 128, 128
NCORES = 8
BSH = B // NCORES          # 4 samples per core
HID = 128                  # MLP hidden
JTOT = COUT * CIN * K * K  # 36864
HW = H * W
RP, CP = H + 2, W + 2      # padded image dims
SCL = 16.0                 # fp8 w2 pre-scale (host mul, folded out via w1/b1)

NPAIRED = 2 * CIN * 3 * COUT   # paired-region w2 cols: (dx,co) x (j,ci)
NSING = CIN * 3 * COUT         # singles-region w2 cols: (dx,co) x ci
NT_P = NPAIRED // 128          # 192 paired chunks
NT_S = NSING // 64             # 192 single chunks
W2CH_P = 4096                  # paired DMA chunk cols (32 mm-chunks)
W2CH_S = 4096                  # singles DMA chunk cols (64 mm-chunks)

XROWS = [(0, 40), (40, 80), (80, 120), (120, 130)]  # x DMA row chunks


def build_kernel_body(nc, tc, ctx, aps):
    x_ap = aps["x"]        # [BSH, CIN, RP, CP] bf16 (host zero-padded)
    w1_ap = aps["w1"]      # [CIN, HID] f32
    b1_ap = aps["b1"]      # [HID, 1] f32 (host /SCL)
    w2_ap = aps["w2"]      # [HID, JTOT] fp8 (host reorder + *SCL)
    b2a_ap = aps["b2a"]    # [128, 192] f32  (j*64+ci, dx*64+co)
    b2b_ap = aps["b2b"]    # [64, 192] f32   (ci, dx*64+co)
    out_ap = aps["out"]    # [BSH, 16, 128, 8, 64] bf16

    const = ctx.enter_context(tc.tile_pool(name="const", bufs=1))
    xpool = ctx.enter_context(tc.tile_pool(name="xpool", bufs=4))
    w2pool = ctx.enter_context(tc.tile_pool(name="w2pool", bufs=9))
    tpool = ctx.enter_context(tc.tile_pool(name="tpool", bufs=2))
    outp = ctx.enter_context(tc.tile_pool(name="outp", bufs=4))
    wg_ps = ctx.enter_context(tc.tile_pool(name="wg_ps", bufs=4, space="PSUM"))
    cv_ps = ctx.enter_context(tc.tile_pool(name="cv_ps", bufs=4, space="PSUM"))

    # ---- x DMA (round-robin over samples per row-chunk) ----
    xd = [xpool.tile([128, RP, CP], BF16, tag="xd", name=f"xd{s}") for s in range(BSH)]
    # tree samples (s2, s3) land first in each round so the DVE trees start
    # early; ACT-accum samples (s0, s1) have more slack
    for r0, r1 in XROWS:
        for s in (2, 0, 3, 1):
            nc.sync.dma_start(
                out=xd[s][0:CIN, r0:r1, :], in_=x_ap[s, :, r0:r1, :]
            )

    # ---- constants (after x, before w2: tiny transfers, needed from ~36us) --
    w1_sb = const.tile([CIN, HID], F32)
    nc.sync.dma_start(out=w1_sb, in_=w1_ap)
    w1s = const.tile([CIN, HID], F32)
    nc.scalar.mul(out=w1s, in_=w1_sb, mul=1.0 / (HW * SCL))
    b1_sb = const.tile([HID, 1], F32)
    nc.sync.dma_start(out=b1_sb, in_=b1_ap)
    b2a = const.tile([128, 192], F32)
    nc.sync.dma_start(out=b2a, in_=b2a_ap)
    b2b = const.tile([CIN, 192], F32)
    nc.sync.dma_start(out=b2b, in_=b2b_ap)

    # ---- w2 DMA (paired region then singles) ----
    w2p_tiles = []
    for c in range(NPAIRED // W2CH_P):  # 6
        t = w2pool.tile([HID, W2CH_P], F8, tag="w2", name=f"w2p{c}")
        nc.sync.dma_start(
            out=t, in_=w2_ap[:, c * W2CH_P : (c + 1) * W2CH_P]
        )
        w2p_tiles.append(t)
    w2s_tiles = []
    for c in range(NSING // W2CH_S):  # 3
        t = w2pool.tile([HID, W2CH_S], F8, tag="w2", name=f"w2s{c}")
        nc.sync.dma_start(
            out=t,
            in_=w2_ap[:, NPAIRED + c * W2CH_S : NPAIRED + (c + 1) * W2CH_S],
        )
        w2s_tiles.append(t)

    # ---- feat + dup per sample ----
    feat4 = const.tile([CIN, BSH], F32)
    # s0: ACT copy+accum -- dup copy is also the feat reduction
    fp0 = const.tile([CIN, len(XROWS)], F32)
    for c, (r0, r1) in enumerate(XROWS):
        d0 = max(0, r0 - 1)
        d1 = r1 - 1
        nc.scalar.activation(
            out=xd[0][CIN : 2 * CIN, d0:d1, :],
            in_=xd[0][0:CIN, d0 + 1 : d1 + 1, :],
            func=mybir.ActivationFunctionType.Copy,
            accum_out=fp0[:, c : c + 1],
        )
    # s3 chunk 0: ACT copy+accum (dup+feat together; writes fpz col 1)
    fpz = const.tile([CIN, 2], F32)
    r0, r1 = XROWS[0]
    nc.scalar.activation(
        out=xd[3][CIN : 2 * CIN, 0 : r1 - 1, :],
        in_=xd[3][0:CIN, 1:r1, :],
        func=mybir.ActivationFunctionType.Copy,
        accum_out=fpz[:, 1:2],
    )
    # s1: feat via DVE cascade below; dup via one DVE 4x copy after the MLP
    # s2+s3: Pool dup copies (feat via DVE trees below); s3's conv is last,
    # so its dup has until ~100us
    for sd, c0 in ((2, 0), (3, 1)):
        for c, (r0, r1) in enumerate(XROWS):
            if c < c0:
                continue
            d0 = max(0, r0 - 1)
            d1 = r1 - 1
            nc.gpsimd.tensor_copy(
                out=xd[sd][CIN : 2 * CIN, d0:d1, :],
                in_=xd[sd][0:CIN, d0 + 1 : d1 + 1, :],
            )
    # s1-3 feat: per-chunk bf16 halving cascade L1->L2->L3 on DVE (chases the
    # x DMA), L3 outputs concatenate per sample, one final reduce each
    def chunk_depth(n0):
        # halving levels until the width goes odd (keeps the cascade exact)
        d = 0
        while n0 % 2 == 0 and d < 3:
            n0 //= 2
            d += 1
        return d, n0

    cat_off = []
    off = 0
    for r0, r1 in XROWS:
        d, nf = chunk_depth((r1 - r0) * CP)
        cat_off.append(off)
        off += nf
    t3cat = {
        s: const.tile([CIN, off], BF16, name=f"t3cat{s}") for s in (1, 2, 3)
    }
    s3w = 0
    for c, (r0, r1) in enumerate(XROWS):
        n0 = (r1 - r0) * CP
        depth, nf = chunk_depth(n0)
        for s in ((1, 2) if c == 0 else (1, 2, 3)):
            o0 = cat_off[c] if s != 3 else s3w
            cur = xd[s][0:CIN, r0:r1, :].rearrange("p r c -> p (r c)")
            n = n0
            for lvl in range(depth):
                n //= 2
                if lvl == depth - 1:
                    dst = t3cat[s][:, o0 : o0 + n]
                else:
                    dst = tpool.tile(
                        [CIN, n], BF16, tag=f"tr{lvl}", name=f"t{lvl}_{s}_{c}"
                    )
                nc.vector.tensor_tensor(
                    out=dst, in0=cur[:, 0:n], in1=cur[:, n : 2 * n],
                    op=mybir.AluOpType.add,
                )
                cur = dst
        if c > 0:
            s3w += nf
    # finals: s1 on ACT (accum to scratch), s2/s3 on DVE, s0 from fp0
    sscr = tpool.tile([CIN, off], BF16, tag="tr0", name="s1scr")
    nc.scalar.activation(
        out=sscr, in_=t3cat[1], func=mybir.ActivationFunctionType.Copy,
        accum_out=feat4[:, 1:2],
    )
    nc.vector.tensor_reduce(
        out=feat4[:, 2:3], in_=t3cat[2], axis=mybir.AxisListType.X,
        op=mybir.AluOpType.add,
    )
    s3scr = tpool.tile([CIN, off], BF16, tag="tr0", name="s3scr")
    nc.scalar.activation(
        out=s3scr[:, 0:s3w], in_=t3cat[3][:, 0:s3w],
        func=mybir.ActivationFunctionType.Copy,
        accum_out=fpz[:, 0:1],
    )
    nc.vector.tensor_reduce(
        out=feat4[:, 3:4], in_=fpz, axis=mybir.AxisListType.X,
        op=mybir.AluOpType.add,
    )
    nc.vector.tensor_reduce(
        out=feat4[:, 0:1], in_=fp0, axis=mybir.AxisListType.X,
        op=mybir.AluOpType.add,
    )

    # ---- MLP ----
    mlp_ps = wg_ps.tile([HID, BSH], F32, tag="wgps", name="mlp_ps")
    nc.tensor.matmul(out=mlp_ps, lhsT=w1s, rhs=feat4, start=True, stop=True)
    h1T = const.tile([HID, BSH], BF16)
    nc.scalar.activation(
        out=h1T, in_=mlp_ps, func=mybir.ActivationFunctionType.Relu,
        bias=b1_sb, scale=1.0,
    )

    # ---- wgen: w2 chunks stationary, h1 moving; psum lands in conv layout --
    psA = [
        wg_ps.tile([128, 512], F32, tag="wgps", name="psA0"),
        wg_ps.tile([128, 512], F32, tag="wgps", name="psA1"),
    ]
    psB = [
        wg_ps.tile([CIN, 512], F32, tag="wgps", name="psB0"),
        wg_ps.tile([CIN, 512], F32, tag="wgps", name="psB1"),
    ]
    for t in range(NT_P):  # paired: chunk t = dx*64+co, partitions (j,ci)
        buf = w2p_tiles[t // 32]
        off = (t % 32) * 128
        dst = psA[t // 128]
        j = (t % 128) * 4
        nc.tensor.matmul(
            out=dst[:, j : j + 4], lhsT=buf[:, off : off + 128], rhs=h1T,
            start=True, stop=True,
        )
    for u in range(NT_S):  # singles: chunk u = dx*64+co, partitions ci
        buf = w2s_tiles[u // 64]
        off = (u % 64) * 64
        dst = psB[u // 128]
        j = (u % 128) * 4
        nc.tensor.matmul(
            out=dst[:, j : j + 4], lhsT=buf[:, off : off + 64], rhs=h1T,
            start=True, stop=True,
        )

    # ---- wt assembly: bf16 wt = psum + b2 (DVE), conv-ready layout ----
    # wtab[s]: cols 0:192 = wtA [(j,ci), (dx,co)]; cols 192:384 rows 64:128
    #          = wtB [(ci)@base64, (dx,co)]
    wtab = [const.tile([128, 384], BF16, name=f"wtab{s}") for s in range(BSH)]
    pa0 = psA[0].rearrange("p (t f) -> p t f", f=4)
    pa1 = psA[1].rearrange("p (t f) -> p t f", f=4)
    pb0 = psB[0].rearrange("p (t f) -> p t f", f=4)
    pb1 = psB[1].rearrange("p (t f) -> p t f", f=4)

    def wt_adds(s):
        nc.vector.tensor_tensor(
            out=wtab[s][:, 0:128], in0=pa0[:, :, s], in1=b2a[:, 0:128],
            op=mybir.AluOpType.add,
        )
        nc.vector.tensor_tensor(
            out=wtab[s][:, 128:192], in0=pa1[:, 0:64, s], in1=b2a[:, 128:192],
            op=mybir.AluOpType.add,
        )
        nc.vector.tensor_tensor(
            out=wtab[s][CIN : 2 * CIN, 192:320], in0=pb0[:, :, s],
            in1=b2b[:, 0:128], op=mybir.AluOpType.add,
        )
        nc.vector.tensor_tensor(
            out=wtab[s][CIN : 2 * CIN, 320:384], in0=pb1[:, 0:64, s],
            in1=b2b[:, 128:192], op=mybir.AluOpType.add,
        )

    # wt5[s]: [(dx0 | dx2) x ci, co] for the s2/s3 five-pass conv
    wt5 = {sa: const.tile([128, COUT], BF16, name=f"wt5_{sa}") for sa in (2, 3)}

    def wt5_adds(sa):
        nc.vector.tensor_tensor(
            out=wt5[sa][0:CIN, :], in0=pb0[:, 0:64, sa], in1=b2b[:, 0:64],
            op=mybir.AluOpType.add,
        )
        nc.vector.tensor_tensor(
            out=wt5[sa][CIN : 2 * CIN, :], in0=pb1[:, 0:64, sa],
            in1=b2b[:, 128:192], op=mybir.AluOpType.add,
        )

    for s in range(BSH):
        wt_adds(s)
    for sa in (2, 3):
        wt5_adds(sa)
    # s1 dup: chunked DVE 4x copies (deadline: s1's conv; chunked so a single
    # long copy can't sit ahead of short critical ops in the ready queue)
    for c, (r0, r1) in enumerate(XROWS):
        d0 = max(0, r0 - 1)
        d1 = r1 - 1
        nc.vector.tensor_copy(
            out=xd[1][CIN : 2 * CIN, d0:d1, :],
            in_=xd[1][0:CIN, d0 + 1 : d1 + 1, :],
        )

    # ---- conv ----
    # s2/s3 get a second dup tile j2 = [j1 | j1 shifted +2 cols] built (DVE,
    # during s0/s1's conv) into the SBUF slots recycled from xd0/xd1; this
    # pairs (dy2,dx0)+(dy2,dx2) into one K=128 pass -> 5 passes instead of 6
    j2 = {}
    YR = 8  # rows per psum bank
    NYB = H // YR
    for s in range(BSH):
        if s >= 2:
            j = xpool.tile([128, RP, CP], BF16, tag="xd", name=f"j2_{s}")
            j2[s] = j
            nc.vector.tensor_copy(
                out=j[0:CIN, 1:129, 0:128],
                in_=xd[s][CIN : 2 * CIN, 1:129, 0:128],
            )
            nc.vector.tensor_copy(
                out=j[CIN : 2 * CIN, 1:129, 0:128],
                in_=xd[s][CIN : 2 * CIN, 1:129, 2:130],
            )
        for yb in range(NYB):
            cvp = cv_ps.tile([128, YR * COUT], F32, tag="cvp", name=f"cvp{s}_{yb}")
            for yr in range(YR):
                y = yb * YR + yr
                sl = cvp[:, yr * COUT : (yr + 1) * COUT]
                for dx in (0, 1, 2):
                    nc.tensor.matmul(
                        out=sl,
                        lhsT=xd[s][:, y, dx : dx + 128],
                        rhs=wtab[s][:, 64 * dx : 64 * dx + 64],
                        start=(dx == 0), stop=False,
                    )
                if s >= 2:
                    nc.tensor.matmul(
                        out=sl,
                        lhsT=j2[s][:, y + 1, 0:128],
                        rhs=wt5[s],
                        start=False, stop=False,
                    )
                    nc.tensor.matmul(
                        out=sl,
                        lhsT=xd[s][CIN : 2 * CIN, y + 1, 1:129],
                        rhs=wtab[s][CIN : 2 * CIN, 256:320],
                        start=False, stop=True,
                    )
                else:
                    for i, dx in enumerate((0, 1, 2)):
                        nc.tensor.matmul(
                            out=sl,
                            lhsT=xd[s][CIN : 2 * CIN, y + 1, dx : dx + 128],
                            rhs=wtab[s][CIN : 2 * CIN, 192 + 64 * dx : 256 + 64 * dx],
                            start=False, stop=(i == 2),
                        )
            ost = outp.tile([128, YR * COUT], BF16, tag="ost", name=f"ost{s}_{yb}")
            nc.scalar.copy(out=ost, in_=cvp)
            nc.sync.dma_start(out=out_ap[s, yb], in_=ost)


_CACHE = {}


def build_nc():
    if "nc" in _CACHE:
        return _CACHE["nc"], _CACHE["aps"]
    nc = bacc.Bacc("TRN2", debug=False, num_devices=NCORES)
    aps = {
        "x": nc.dram_tensor("x", [BSH, CIN, RP, CP], BF16, kind="ExternalInput").ap(),
        "w1": nc.dram_tensor("w1", [CIN, HID], F32, kind="ExternalInput").ap(),
        "b1": nc.dram_tensor("b1", [HID, 1], F32, kind="ExternalInput").ap(),
        "w2": nc.dram_tensor("w2", [HID, JTOT], F8, kind="ExternalInput").ap(),
        "b2a": nc.dram_tensor("b2a", [128, 192], F32, kind="ExternalInput").ap(),
        "b2b": nc.dram_tensor("b2b", [CIN, 192], F32, kind="ExternalInput").ap(),
        "out": nc.dram_tensor(
            "out", [BSH, H // 8, 128, 8, COUT], BF16, kind="ExternalOutput"
        ).ap(),
    }
    with tile.TileContext(nc) as tc, ExitStack() as ctx:
        build_kernel_body(nc, tc, ctx, aps)
    nc.compile()
    _CACHE["nc"] = nc
    _CACHE["aps"] = aps
    return nc, aps


def make_in_maps(x, w1, b1, w2, b2):
    import ml_dtypes

    x = np.asarray(x, dtype=np.float32)
    xpad = np.zeros((B, CIN, RP, CP), dtype=ml_dtypes.bfloat16)
    xpad[:, :, 1 : H + 1, 1 : W + 1] = x.astype(ml_dtypes.bfloat16)
    w1 = np.ascontiguousarray(np.asarray(w1, dtype=np.float32))
    b1 = np.ascontiguousarray(
        np.asarray(b1, dtype=np.float32).reshape(HID, 1) / SCL
    )
    # w2 reorder: paired cols (dx,co)x(j,ci) from o=3j+dx; singles from o=6+dx
    w2v = (np.asarray(w2, dtype=np.float32) * SCL).reshape(HID, COUT, CIN, 9)
    w2p = w2v[:, :, :, [3 * j + dx for dx in range(3) for j in range(2)]]
    # -> [HID, co, ci, (dx,j)] want cols ((dx,co),(j,ci))
    w2p = w2p.reshape(HID, COUT, CIN, 3, 2).transpose(0, 3, 1, 4, 2)
    w2p = w2p.reshape(HID, NPAIRED)
    w2s = w2v[:, :, :, [6 + dx for dx in range(3)]].transpose(0, 3, 1, 2)
    w2s = w2s.reshape(HID, NSING)
    w2x = np.ascontiguousarray(
        np.concatenate([w2p, w2s], axis=1).astype(ml_dtypes.float8_e4m3fn)
    )
    b2v = np.asarray(b2, dtype=np.float32).reshape(COUT, CIN, 9)
    b2a = np.zeros((128, 192), dtype=np.float32)
    b2bt = np.zeros((CIN, 192), dtype=np.float32)
    for dx in range(3):
        for j in range(2):
            # b2a[j*64+ci, dx*64+co] = b2[co, ci, 3j+dx]
            b2a[j * CIN : (j + 1) * CIN, dx * COUT : (dx + 1) * COUT] = b2v[
                :, :, 3 * j + dx
            ].T
        b2bt[:, dx * COUT : (dx + 1) * COUT] = b2v[:, :, 6 + dx].T
    in_maps = []
    for c in range(NCORES):
        in_maps.append(
            {
                "x": np.ascontiguousarray(xpad[c * BSH : (c + 1) * BSH]),
                "w1": w1,
                "b1": b1,
                "w2": w2x,
                "b2a": np.ascontiguousarray(b2a),
                "b2b": np.ascontiguousarray(b2bt),
            }
        )
    return in_maps


def kernel(x, w1, b1, w2, b2, _trace=False, _results_out=None):
    nc, _ = build_nc()
    in_maps = make_in_maps(x, w1, b1, w2, b2)
    res = run_bass_kernel_spmd(
        nc, in_maps, core_ids=list(range(NCORES)), trace=_trace
    )
    if _results_out is not None:
        _results_out.append(res)
    # [BSH,16,128,8,64] bf16 per core -> [B, COUT, H, W] f32
    parts = []
    for r in res.results:
        o = np.asarray(r["out"], dtype=np.float32)  # [BSH,16,x128,yr8,co64]
        o = o.transpose(0, 4, 1, 3, 2).reshape(BSH, COUT, H, W)
        parts.append(o)
    return np.concatenate(parts, axis=0)


if __name__ == "__main__":
    rng = np.random.default_rng(0)
    ins = {
        "x": rng.standard_normal((B, CIN, H, W)).astype(np.float32),
        "w1": (rng.standard_normal((CIN, HID)) * 0.05).astype(np.float32),
        "b1": (rng.standard_normal((HID,)) * 0.05).astype(np.float32),
        "w2": (rng.standard_normal((HID, JTOT)) * 0.05).astype(np.float32),
        "b2": (rng.standard_normal((JTOT,)) * 0.05).astype(np.float32),
    }
    out = kernel(**ins)
    print("out", out.shape, out.dtype, np.abs(out).mean())


# revision 24
# speedup vs baseline: 1.5861x; 1.0603x over previous
"""Trainium2 Bass kernel for nn_DynamicConv2d: per-sample dynamic conv.

  feat = x.mean(H,W); h1 = relu(feat@w1+b1); wgen = (h1@w2+b2) -> per-sample
  [COUT, CIN, 3, 3] conv weights; out[s] = conv2d(x[s], wgen[s], pad=1).

Sharding: batch B=32 across 8 cores (4 samples/core), MLP params replicated.

Per-core pipeline (v2 -- x-stationary conv):
  - x arrives host zero-padded [4, 64, 130, 130] bf16; per sample an SBUF
    tile xd [128=(j,ci), 130, 130] holds j0 = padded image (DMA) and
    j1[r] = j0[r+1] (row-shifted dup, built by on-chip partition-shifted
    copies on ACT/Pool/DVE, overlapped with the x DMA)
  - feat: s0 via ACT copy+accum (the dup copy doubles as the reduction);
    s1-3 via DVE halving add-trees (level-1 bf16 at 4x, then fp32)
  - MLP: h1 = relu(w1.T/(HW*16) @ feat + b1/16) -> bf16 [128, 4]
  - wgen: w2 host-reordered/scaled(x16)/fp8 as stationary chunks
    [HID, 128]; rhs = h1 [HID, 4] -> psum lands directly in conv-weight
    layout [(dy,ci) | ci, (dx,co)]; DVE adds b2 -> bf16 wt tiles
  - conv: per (sample, row y): 6 matmuls into one psum [128pix, 64co]
    slice: 3 paired passes (K=128 contracts dy=0,1 x ci via the dup) +
    3 singles (dy=2, K=64 on the j1 half); 8 rows/psum bank; ACT drains
    bf16; DMA out in [s, yb, x, yr, co] layout, host restores NCHW fp32.
"""

import sys

for _p in ("/opt/trn_rl_repo",):
    if _p not in sys.path:
        sys.path.insert(0, _p)

from contextlib import ExitStack

import numpy as np

import concourse.bass as bass
import concourse.tile as tile
from concourse import bacc, mybir
from concourse.bass_utils import run_bass_kernel_spmd

F32 = mybir.dt.float32
BF16 = mybir.dt.bfloat16
F8 = mybir.dt.float8e4

B, CIN, COUT, K, H, W = 32, 64, 64, 3, 128, 128
NCORES = 8
BSH = B // NCORES          # 4 samples per core
HID = 128                  # MLP hidden
JTOT = COUT * CIN * K * K  # 36864
HW = H * W
RP, CP = H + 2, W + 2      # padded image dims
SCL = 16.0                 # fp8 w2 pre-scale (host mul, folded out via w1/b1)

NPAIRED = 2 * CIN * 3 * COUT   # paired-region w2 cols: (dx,co) x (j,ci)
NSING = CIN * 3 * COUT         # singles-region w2 cols: (dx,co) x ci
NT_P = NPAIRED // 128          # 192 paired chunks
NT_S = NSING // 64             # 192 single chunks
W2CH_P = 4096                  # paired DMA chunk cols (32 mm-chunks)
W2CH_S = 4096                  # singles DMA chunk cols (64 mm-chunks)

XROWS = [(0, 40), (40, 80), (80, 120), (120, 130)]  # x DMA row chunks


def build_kernel_body(nc, tc, ctx, aps):
    x_ap = aps["x"]        # [BSH, CIN, RP, CP] bf16 (host zero-padded)
    w1_ap = aps["w1"]      # [CIN, HID] f32
    b1_ap = aps["b1"]      # [HID, 1] f32 (host /SCL)
    w2_ap = aps["w2"]      # [HID, JTOT] fp8 (host reorder + *SCL)
    b2a_ap = aps["b2a"]    # [128, 192] f32  (j*64+ci, dx*64+co)
    b2b_ap = aps["b2b"]    # [64, 192] f32   (ci, dx*64+co)
    out_ap = aps["out"]    # [BSH, 16, 128, 8, 64] bf16

    const = ctx.enter_context(tc.tile_pool(name="const", bufs=1))
    xpool = ctx.enter_context(tc.tile_pool(name="xpool", bufs=4))
    w2pool = ctx.enter_context(tc.tile_pool(name="w2pool", bufs=9))
    tpool = ctx.enter_context(tc.tile_pool(name="tpool", bufs=2))
    outp = ctx.enter_context(tc.tile_pool(name="outp", bufs=4))
    wg_ps = ctx.enter_context(tc.tile_pool(name="wg_ps", bufs=4, space="PSUM"))
    cv_ps = ctx.enter_context(tc.tile_pool(name="cv_ps", bufs=4, space="PSUM"))

    # ---- x DMA (round-robin over samples per row-chunk) ----
    xd = [xpool.tile([128, RP, CP], BF16, tag="xd", name=f"xd{s}") for s in range(BSH)]
    # zero pad rows 0 and 129 of j0 on-chip (skipping them in the DMA saves
    # ~0.4us on the serial DMA chain); cascade/feat samples land first
    for s in range(BSH):
        nc.vector.memset(xd[s][0:CIN, 0, :], 0.0)
        nc.vector.memset(xd[s][0:CIN, RP - 1, :], 0.0)
    for r0, r1 in XROWS:
        rr0, rr1 = max(r0, 1), min(r1, RP - 1)
        for s in (2, 0, 3, 1):
            nc.sync.dma_start(
                out=xd[s][0:CIN, rr0:rr1, :], in_=x_ap[s, :, rr0:rr1, :]
            )

    # ---- constants (after x, before w2: tiny transfers, needed from ~36us) --
    w1_sb = const.tile([CIN, HID], F32)
    nc.sync.dma_start(out=w1_sb, in_=w1_ap)
    w1s = const.tile([CIN, HID], F32)
    nc.scalar.mul(out=w1s, in_=w1_sb, mul=1.0 / (HW * SCL))
    b1_sb = const.tile([HID, 1], F32)
    nc.sync.dma_start(out=b1_sb, in_=b1_ap)
    b2a = const.tile([128, 192], F32)
    nc.sync.dma_start(out=b2a, in_=b2a_ap)
    b2b = const.tile([CIN, 192], F32)
    nc.sync.dma_start(out=b2b, in_=b2b_ap)

    # ---- w2 DMA (paired region then singles) ----
    w2p_tiles = []
    for c in range(NPAIRED // W2CH_P):  # 6
        t = w2pool.tile([HID, W2CH_P], F8, tag="w2", name=f"w2p{c}")
        nc.sync.dma_start(
            out=t, in_=w2_ap[:, c * W2CH_P : (c + 1) * W2CH_P]
        )
        w2p_tiles.append(t)
    w2s_tiles = []
    for c in range(NSING // W2CH_S):  # 3
        t = w2pool.tile([HID, W2CH_S], F8, tag="w2", name=f"w2s{c}")
        nc.sync.dma_start(
            out=t,
            in_=w2_ap[:, NPAIRED + c * W2CH_S : NPAIRED + (c + 1) * W2CH_S],
        )
        w2s_tiles.append(t)

    # ---- feat + dup per sample ----
    # ACT: copy+accum chunks for s0 (all) and s1 (c0,c1) -- the dup copy IS
    # the feat reduction. DVE: bf16 halving cascades for s1 (c2,c3), s2, s3.
    # Pool: dup copies for s2, s3. Emission interleaved by chunk arrival.
    feat4 = const.tile([CIN, BSH], F32)
    fp0 = const.tile([CIN, len(XROWS)], F32)
    fp1 = const.tile([CIN, 3], F32)
    for c, (r0, r1) in enumerate(XROWS):
        d0 = max(0, r0 - 1)
        d1 = r1 - 1
        nc.scalar.activation(
            out=xd[0][CIN : 2 * CIN, d0:d1, :],
            in_=xd[0][0:CIN, d0 + 1 : d1 + 1, :],
            func=mybir.ActivationFunctionType.Copy,
            accum_out=fp0[:, c : c + 1],
        )
        if c < 2:
            nc.scalar.activation(
                out=xd[1][CIN : 2 * CIN, d0:d1, :],
                in_=xd[1][0:CIN, d0 + 1 : d1 + 1, :],
                func=mybir.ActivationFunctionType.Copy,
                accum_out=fp1[:, c : c + 1],
            )
    # Pool dup copies for s2, s3
    for sd in (2, 3):
        for c, (r0, r1) in enumerate(XROWS):
            d0 = max(0, r0 - 1)
            d1 = r1 - 1
            nc.gpsimd.tensor_copy(
                out=xd[sd][CIN : 2 * CIN, d0:d1, :],
                in_=xd[sd][0:CIN, d0 + 1 : d1 + 1, :],
            )

    # DVE cascades: per-chunk L1->L2->L3 bf16 halvings into per-sample concat
    def chunk_depth(n0):
        d = 0
        while n0 % 2 == 0 and d < 3:
            n0 //= 2
            d += 1
        return d, n0

    CASC = {1: (2, 3), 2: (0, 1, 2, 3), 3: (0, 1, 2, 3)}
    catw = {
        sa: sum(chunk_depth((r1 - r0) * CP)[1]
                for c, (r0, r1) in enumerate(XROWS) if c in CASC[sa])
        for sa in (1, 2, 3)
    }
    t3cat = {
        sa: const.tile([CIN, catw[sa]], BF16, name=f"t3cat{sa}")
        for sa in (1, 2, 3)
    }
    woff = {1: 0, 2: 0, 3: 0}
    for c, (r0, r1) in enumerate(XROWS):
        n0 = (r1 - r0) * CP
        depth, nf = chunk_depth(n0)
        for sa in (2, 3, 1):
            if c not in CASC[sa]:
                continue
            cur = xd[sa][0:CIN, r0:r1, :].rearrange("p r c -> p (r c)")
            n = n0
            for lvl in range(depth):
                n //= 2
                if lvl == depth - 1:
                    dst = t3cat[sa][:, woff[sa] : woff[sa] + n]
                else:
                    dst = tpool.tile(
                        [CIN, n], BF16, tag=f"tr{lvl}", name=f"t{lvl}_{sa}_{c}"
                    )
                nc.vector.tensor_tensor(
                    out=dst, in0=cur[:, 0:n], in1=cur[:, n : 2 * n],
                    op=mybir.AluOpType.add,
                )
                cur = dst
            woff[sa] += nf
    # finals: s1-part + s3 on ACT (accum), s2 on DVE, s0/s1 combines on DVE
    sscr = tpool.tile([CIN, 2275], BF16, tag="tr0", name="s1scr")
    nc.scalar.activation(
        out=sscr[:, 0 : catw[1]], in_=t3cat[1],
        func=mybir.ActivationFunctionType.Copy,
        accum_out=fp1[:, 2:3],
    )
    nc.vector.tensor_reduce(
        out=feat4[:, 2:3], in_=t3cat[2], axis=mybir.AxisListType.X,
        op=mybir.AluOpType.add,
    )
    s3scr = tpool.tile([CIN, 2275], BF16, tag="tr0", name="s3scr")
    nc.scalar.activation(
        out=s3scr[:, 0 : catw[3]], in_=t3cat[3],
        func=mybir.ActivationFunctionType.Copy,
        accum_out=feat4[:, 3:4],
    )
    nc.vector.tensor_reduce(
        out=feat4[:, 1:2], in_=fp1, axis=mybir.AxisListType.X,
        op=mybir.AluOpType.add,
    )
    nc.vector.tensor_reduce(
        out=feat4[:, 0:1], in_=fp0, axis=mybir.AxisListType.X,
        op=mybir.AluOpType.add,
    )

    # ---- MLP ----
    mlp_ps = wg_ps.tile([HID, BSH], F32, tag="wgps", name="mlp_ps")
    nc.tensor.matmul(out=mlp_ps, lhsT=w1s, rhs=feat4, start=True, stop=True)
    h1T = const.tile([HID, BSH], BF16)
    nc.scalar.activation(
        out=h1T, in_=mlp_ps, func=mybir.ActivationFunctionType.Relu,
        bias=b1_sb, scale=1.0,
    )

    # ---- wgen: w2 chunks stationary, h1 moving; psum lands in conv layout --
    psA = [
        wg_ps.tile([128, 512], F32, tag="wgps", name="psA0"),
        wg_ps.tile([128, 512], F32, tag="wgps", name="psA1"),
    ]
    psB = [
        wg_ps.tile([CIN, 512], F32, tag="wgps", name="psB0"),
        wg_ps.tile([CIN, 512], F32, tag="wgps", name="psB1"),
    ]
    for t in range(NT_P):  # paired: chunk t = dx*64+co, partitions (j,ci)
        buf = w2p_tiles[t // 32]
        off = (t % 32) * 128
        dst = psA[t // 128]
        j = (t % 128) * 4
        nc.tensor.matmul(
            out=dst[:, j : j + 4], lhsT=buf[:, off : off + 128], rhs=h1T,
            start=True, stop=True,
        )
    for u in range(NT_S):  # singles: chunk u = dx*64+co, partitions ci
        buf = w2s_tiles[u // 64]
        off = (u % 64) * 64
        dst = psB[u // 128]
        j = (u % 128) * 4
        nc.tensor.matmul(
            out=dst[:, j : j + 4], lhsT=buf[:, off : off + 64], rhs=h1T,
            start=True, stop=True,
        )

    # ---- wt assembly: bf16 wt = psum + b2 (DVE), conv-ready layout ----
    # wtab[s]: cols 0:192 = wtA [(j,ci), (dx,co)]; cols 192:384 rows 64:128
    #          = wtB [(ci)@base64, (dx,co)]
    wtab = [const.tile([128, 384], BF16, name=f"wtab{s}") for s in range(BSH)]
    pa0 = psA[0].rearrange("p (t f) -> p t f", f=4)
    pa1 = psA[1].rearrange("p (t f) -> p t f", f=4)
    pb0 = psB[0].rearrange("p (t f) -> p t f", f=4)
    pb1 = psB[1].rearrange("p (t f) -> p t f", f=4)

    def wt_adds(s):
        nc.vector.tensor_tensor(
            out=wtab[s][:, 0:128], in0=pa0[:, :, s], in1=b2a[:, 0:128],
            op=mybir.AluOpType.add,
        )
        nc.vector.tensor_tensor(
            out=wtab[s][:, 128:192], in0=pa1[:, 0:64, s], in1=b2a[:, 128:192],
            op=mybir.AluOpType.add,
        )
        nc.vector.tensor_tensor(
            out=wtab[s][CIN : 2 * CIN, 192:320], in0=pb0[:, :, s],
            in1=b2b[:, 0:128], op=mybir.AluOpType.add,
        )
        nc.vector.tensor_tensor(
            out=wtab[s][CIN : 2 * CIN, 320:384], in0=pb1[:, 0:64, s],
            in1=b2b[:, 128:192], op=mybir.AluOpType.add,
        )

    # wt5[s]: [(dx0 | dx2) x ci, co] for the s2/s3 five-pass conv
    wt5 = {sa: const.tile([128, COUT], BF16, name=f"wt5_{sa}") for sa in (2, 3)}

    def wt5_adds(sa):
        nc.vector.tensor_tensor(
            out=wt5[sa][0:CIN, :], in0=pb0[:, 0:64, sa], in1=b2b[:, 0:64],
            op=mybir.AluOpType.add,
        )
        nc.vector.tensor_tensor(
            out=wt5[sa][CIN : 2 * CIN, :], in0=pb1[:, 0:64, sa],
            in1=b2b[:, 128:192], op=mybir.AluOpType.add,
        )

    for s in range(BSH):
        wt_adds(s)
    for sa in (2, 3):
        wt5_adds(sa)
    # s1 dup (chunks 2-3; 0-1 done by the ACT accum-copies): DVE 4x copies
    for c, (r0, r1) in enumerate(XROWS[2:], start=2):
        d0 = max(0, r0 - 1)
        d1 = r1 - 1
        nc.vector.tensor_copy(
            out=xd[1][CIN : 2 * CIN, d0:d1, :],
            in_=xd[1][0:CIN, d0 + 1 : d1 + 1, :],
        )

    # ---- conv ----
    # s2/s3 get a second dup tile j2 = [j1 | j1 shifted +2 cols] built (DVE,
    # during s0/s1's conv) into the SBUF slots recycled from xd0/xd1; this
    # pairs (dy2,dx0)+(dy2,dx2) into one K=128 pass -> 5 passes instead of 6
    j2 = {}
    YR = 8  # rows per psum bank
    NYB = H // YR
    for s in range(BSH):
        if s >= 2:
            j = xpool.tile([128, RP, CP], BF16, tag="xd", name=f"j2_{s}")
            j2[s] = j
            nc.vector.tensor_copy(
                out=j[0:CIN, 1:129, 0:128],
                in_=xd[s][CIN : 2 * CIN, 1:129, 0:128],
            )
            nc.vector.tensor_copy(
                out=j[CIN : 2 * CIN, 1:129, 0:128],
                in_=xd[s][CIN : 2 * CIN, 1:129, 2:130],
            )
        groups = [(yb * YR, YR) for yb in range(NYB)]
        if s == 1:  # last conv sample: end with a tiny bank -> short tail
            groups = groups[:-1] + [(H - YR, YR - 2), (H - 2, 2)]
        for y0, nyr in groups:  # groups never cross a yb boundary
            cvp = cv_ps.tile([128, YR * COUT], F32, tag="cvp", name=f"cvp{s}_{y0}")
            for yr in range(nyr):
                y = y0 + yr
                sl = cvp[:, yr * COUT : (yr + 1) * COUT]
                for dx in (0, 1, 2):
                    nc.tensor.matmul(
                        out=sl,
                        lhsT=xd[s][:, y, dx : dx + 128],
                        rhs=wtab[s][:, 64 * dx : 64 * dx + 64],
                        start=(dx == 0), stop=False,
                    )
                if s >= 2:
                    nc.tensor.matmul(
                        out=sl,
                        lhsT=j2[s][:, y + 1, 0:128],
                        rhs=wt5[s],
                        start=False, stop=False,
                    )
                    nc.tensor.matmul(
                        out=sl,
                        lhsT=xd[s][CIN : 2 * CIN, y + 1, 1:129],
                        rhs=wtab[s][CIN : 2 * CIN, 256:320],
                        start=False, stop=True,
                    )
                else:
                    for i, dx in enumerate((0, 1, 2)):
                        nc.tensor.matmul(
                            out=sl,
                            lhsT=xd[s][CIN : 2 * CIN, y + 1, dx : dx + 128],
                            rhs=wtab[s][CIN : 2 * CIN, 192 + 64 * dx : 256 + 64 * dx],
                            start=False, stop=(i == 2),
                        )
            ost = outp.tile([128, YR * COUT], BF16, tag="ost", name=f"ost{s}_{y0}")
            nc.scalar.copy(
                out=ost[:, 0 : nyr * COUT], in_=cvp[:, 0 : nyr * COUT]
            )
            nc.sync.dma_start(
                out=out_ap[s, y0 // YR, :, y0 % YR : y0 % YR + nyr],
                in_=ost[:, 0 : nyr * COUT],
            )


_CACHE = {}


def build_nc():
    if "nc" in _CACHE:
        return _CACHE["nc"], _CACHE["aps"]
    nc = bacc.Bacc("TRN2", debug=False, num_devices=NCORES)
    aps = {
        "x": nc.dram_tensor("x", [BSH, CIN, RP, CP], BF16, kind="ExternalInput").ap(),
        "w1": nc.dram_tensor("w1", [CIN, HID], F32, kind="ExternalInput").ap(),
        "b1": nc.dram_tensor("b1", [HID, 1], F32, kind="ExternalInput").ap(),
        "w2": nc.dram_tensor("w2", [HID, JTOT], F8, kind="ExternalInput").ap(),
        "b2a": nc.dram_tensor("b2a", [128, 192], F32, kind="ExternalInput").ap(),
        "b2b": nc.dram_tensor("b2b", [CIN, 192], F32, kind="ExternalInput").ap(),
        "out": nc.dram_tensor(
            "out", [BSH, H // 8, 128, 8, COUT], BF16, kind="ExternalOutput"
        ).ap(),
    }
    with tile.TileContext(nc) as tc, ExitStack() as ctx:
        build_kernel_body(nc, tc, ctx, aps)
    nc.compile()
    _CACHE["nc"] = nc
    _CACHE["aps"] = aps
    return nc, aps


def make_in_maps(x, w1, b1, w2, b2):
    import ml_dtypes

    x = np.asarray(x, dtype=np.float32)
    xpad = np.zeros((B, CIN, RP, CP), dtype=ml_dtypes.bfloat16)
    xpad[:, :, 1 : H + 1, 1 : W + 1] = x.astype(ml_dtypes.bfloat16)
    w1 = np.ascontiguousarray(np.asarray(w1, dtype=np.float32))
    b1 = np.ascontiguousarray(
        np.asarray(b1, dtype=np.float32).reshape(HID, 1) / SCL
    )
    # w2 reorder: paired cols (dx,co)x(j,ci) from o=3j+dx; singles from o=6+dx
    w2v = (np.asarray(w2, dtype=np.float32) * SCL).reshape(HID, COUT, CIN, 9)
    w2p = w2v[:, :, :, [3 * j + dx for dx in range(3) for j in range(2)]]
    # -> [HID, co, ci, (dx,j)] want cols ((dx,co),(j,ci))
    w2p = w2p.reshape(HID, COUT, CIN, 3, 2).transpose(0, 3, 1, 4, 2)
    w2p = w2p.reshape(HID, NPAIRED)
    w2s = w2v[:, :, :, [6 + dx for dx in range(3)]].transpose(0, 3, 1, 2)
    w2s = w2s.reshape(HID, NSING)
    w2x = np.ascontiguousarray(
        np.concatenate([w2p, w2s], axis=1).astype(ml_dtypes.float8_e4m3fn)
    )
    b2v = np.asarray(b2, dtype=np.float32).reshape(COUT, CIN, 9)
    b2a = np.zeros((128, 192), dtype=np.float32)
    b2bt = np.zeros((CIN, 192), dtype=np.float32)
    for dx in range(3):
        for j in range(2):
            # b2a[j*64+ci, dx*64+co] = b2[co, ci, 3j+dx]
            b2a[j * CIN : (j + 1) * CIN, dx * COUT : (dx + 1) * COUT] = b2v[
                :, :, 3 * j + dx
            ].T
        b2bt[:, dx * COUT : (dx + 1) * COUT] = b2v[:, :, 6 + dx].T
    in_maps = []
    for c in range(NCORES):
        in_maps.append(
            {
                "x": np.ascontiguousarray(xpad[c * BSH : (c + 1) * BSH]),
                "w1": w1,
                "b1": b1,
                "w2": w2x,
                "b2a": np.ascontiguousarray(b2a),
                "b2b": np.ascontiguousarray(b2bt),
            }
        )
    return in_maps


def kernel(x, w1, b1, w2, b2, _trace=False, _results_out=None):
    nc, _ = build_nc()
    in_maps = make_in_maps(x, w1, b1, w2, b2)
    res = run_bass_kernel_spmd(
        nc, in_maps, core_ids=list(range(NCORES)), trace=_trace
    )
    if _results_out is not None:
        _results_out.append(res)
    # [BSH,16,128,8,64] bf16 per core -> [B, COUT, H, W] f32
    parts = []
    for r in res.results:
        o = np.asarray(r["out"], dtype=np.float32)  # [BSH,16,x128,yr8,co64]
        o = o.transpose(0, 4, 1, 3, 2).reshape(BSH, COUT, H, W)
        parts.append(o)
    return np.concatenate(parts, axis=0)


if __name__ == "__main__":
    rng = np.random.default_rng(0)
    ins = {
        "x": rng.standard_normal((B, CIN, H, W)).astype(np.float32),
        "w1": (rng.standard_normal((CIN, HID)) * 0.05).astype(np.float32),
        "b1": (rng.standard_normal((HID,)) * 0.05).astype(np.float32),
        "w2": (rng.standard_normal((HID, JTOT)) * 0.05).astype(np.float32),
        "b2": (rng.standard_normal((JTOT,)) * 0.05).astype(np.float32),
    }
    out = kernel(**ins)
    print("out", out.shape, out.dtype, np.abs(out).mean())
